# revision 8
# baseline (speedup 1.0000x reference)
"""GAT message-passing kernel for Trainium2 (8 NeuronCores, SPMD).

Problem (per full input):
    B=8, S=512, N=32 neighbors, H=256, V=100001
    out[b,s,:] = sum_n softmax_n(leakyrelu(a_w . [src, cand_n]) + mask*NEG) * cand_n
    candidates = [self] + 32 neighbors (self never masked)

Sharding: data-parallel over B — core c handles batch row c with a
replicated embedding table.

Per-core algorithm (s-tiles of 128 nodes, 4 tiles):
    - build candidate id tile [128, 33] (col 0 = self, 1..32 = neighbors)
    - ONE indirect DMA gathers 33 rows of 256 f32 per partition:
      F[s][n,h], rows contiguous per partition (DMA-friendly 1KB rows)
    - logits: z[s,n] = a_src.src[s] + a_cand.cand[s,n] + a_b computed as
      fused tensor_tensor_reduce ops (mult + add-reduce with init scalar)
    - leakyrelu, neighbor masking, softmax over 33 slots
      (exp + denominator in ONE ScalarE op via accum_out)
    - weighted sum on the TensorEngine: out = sum_n diag(w[:,n]) @ F_n
      accumulated in PSUM across the 33 candidate slots
"""

import numpy as np

B, S, N, H, V = 8, 512, 32, 256, 100001
NC1 = N + 1  # 33 candidate slots (self + neighbors)
P = 128
S_TILES = S // P
NEG = -1.0e9
SLOPE = 0.2
N_CORES = 8

# Tuning knobs
F_BUFS = 2  # gather-tile double buffering (F and Pbuf share the pool)

_CACHE: dict = {}


def _build_nc():
    import concourse.bacc as bacc
    import concourse.mybir as mybir
    import concourse.tile as tile
    from concourse import bass
    from concourse.masks import make_identity

    f32 = mybir.dt.float32
    i32 = mybir.dt.int32
    Alu = mybir.AluOpType
    Act = mybir.ActivationFunctionType
    X = mybir.AxisListType.X

    nc = bacc.Bacc(
        "TRN2",
        target_bir_lowering=False,
        debug=False,
        enable_asserts=False,
        num_devices=N_CORES,
    )

    ids_d = nc.dram_tensor("node_ids", [S, 1], i32, kind="ExternalInput").ap()
    nbr_d = nc.dram_tensor("neighs", [S, N], i32, kind="ExternalInput").ap()
    msk_d = nc.dram_tensor("mask", [S, N], i32, kind="ExternalInput").ap()
    emb_d = nc.dram_tensor("emb_table", [V, H], f32, kind="ExternalInput").ap()
    aw_d = nc.dram_tensor("a_w", [2, H], f32, kind="ExternalInput").ap()
    ab_d = nc.dram_tensor("a_b", [1, 1], f32, kind="ExternalInput").ap()
    out_d = nc.dram_tensor("out", [S, H], f32, kind="ExternalOutput").ap()

    with tile.TileContext(nc) as tc:
        with (
            tc.tile_pool(name="cpool", bufs=1) as cpool,
            tc.tile_pool(name="fpool", bufs=F_BUFS) as fpool,
            tc.tile_pool(name="spool", bufs=2) as spool,
            tc.tile_pool(name="dpool", bufs=8) as dpool,
            tc.tile_pool(name="ppool", bufs=2, space="PSUM") as ppool,
        ):
            # ---- constants (once) ----
            ident = cpool.tile([P, P], f32)
            make_identity(nc, ident)

            # replicate a_w rows (a_src = row 0, a_cand = row 1) to all
            # 128 partitions with a tiny indirect gather
            # one offset (row 0) per partition; 512 contiguous floats cover
            # both a_w rows -> [aws | awc] per partition
            aw_idx = cpool.tile([P, 1], i32)
            nc.gpsimd.memset(aw_idx[:], 0)
            aw_rep = cpool.tile([P, 2 * H], f32)
            nc.gpsimd.indirect_dma_start(
                out=aw_rep[:],
                out_offset=None,
                in_=aw_d,
                in_offset=bass.IndirectOffsetOnAxis(ap=aw_idx[:], axis=0),
            )
            aws_rep = aw_rep[:, 0:H]
            awc_rep = aw_rep[:, H : 2 * H]

            ab_idx = cpool.tile([P, 1], i32)
            nc.gpsimd.memset(ab_idx[:], 0)
            ab_rep = cpool.tile([P, 1], f32)
            nc.gpsimd.indirect_dma_start(
                out=ab_rep[:],
                out_offset=None,
                in_=ab_d,
                in_offset=bass.IndirectOffsetOnAxis(ap=ab_idx[:], axis=0),
            )

            for t in range(S_TILES):
                rows = slice(t * P, (t + 1) * P)

                idx = spool.tile([P, NC1], i32)
                nc.sync.dma_start(out=idx[:, 0:1], in_=ids_d[rows, :])
                nc.sync.dma_start(out=idx[:, 1:NC1], in_=nbr_d[rows, :])
                mask_i = spool.tile([P, N], i32)
                nc.sync.dma_start(out=mask_i[:], in_=msk_d[rows, :])

                F = fpool.tile([P, NC1 * H], f32)
                F3 = F.rearrange("p (n h) -> p n h", n=NC1)
                # HW contract: one offset per partition per indirect DMA
                for n in range(NC1):
                    nc.gpsimd.indirect_dma_start(
                        out=F3[:, n, :],
                        out_offset=None,
                        in_=emb_d,
                        in_offset=bass.IndirectOffsetOnAxis(
                            ap=idx[:, n : n + 1], axis=0
                        ),
                    )

                # ---- logits ----
                # z[s,n] = sum_h F[s,n,h]*awc[h] + (sum_h F[s,0,h]*aws[h] + ab)
                trash = spool.tile([P, H], f32)
                zsrc = spool.tile([P, 1], f32)
                nc.vector.tensor_mul(trash[:], F3[:, 0, :], aws_rep)
                nc.vector.tensor_reduce(zsrc[:], trash[:], axis=X, op=Alu.add)
                nc.vector.tensor_scalar_add(zsrc[:], zsrc[:], ab_rep[:])
                Pbuf = fpool.tile([P, NC1 * H], f32, name="Pbuf")
                P3 = Pbuf.rearrange("p (n h) -> p n h", n=NC1)
                nc.vector.tensor_mul(
                    P3, F3, awc_rep.unsqueeze(1).to_broadcast([P, NC1, H])
                )
                z = spool.tile([P, NC1], f32)
                nc.vector.tensor_reduce(z[:], P3, axis=X, op=Alu.add)
                nc.vector.tensor_scalar_add(z[:], z[:], zsrc[:])

                # ---- leakyrelu ----
                z2 = spool.tile([P, NC1], f32)
                nc.vector.tensor_scalar_mul(z2[:], z[:], SLOPE)
                nc.vector.tensor_max(z[:], z[:], z2[:])

                # ---- mask neighbors: z[:,1:] += mask * NEG ----
                mask_f = spool.tile([P, N], f32)
                nc.vector.tensor_copy(mask_f[:], mask_i[:])
                nc.vector.scalar_tensor_tensor(
                    out=z[:, 1:NC1],
                    in0=mask_f[:],
                    scalar=NEG,
                    in1=z[:, 1:NC1],
                    op0=Alu.mult,
                    op1=Alu.add,
                )

                # ---- softmax over 33 slots ----
                negm = spool.tile([P, 1], f32)
                nc.vector.tensor_reduce(
                    negm[:], z[:], axis=X, op=Alu.max, negate=True
                )
                e = spool.tile([P, NC1], f32)
                den = spool.tile([P, 1], f32)
                nc.scalar.activation(
                    e[:], z[:], Act.Exp, bias=negm[:], scale=1.0, accum_out=den[:]
                )
                rden = spool.tile([P, 1], f32)
                nc.vector.reciprocal(rden[:], den[:])
                w = spool.tile([P, NC1], f32)
                nc.vector.tensor_scalar_mul(w[:], e[:], rden[:])

                # ---- weighted aggregation on TensorE ----
                acc = ppool.tile([P, H], f32)
                for n in range(NC1):
                    dg = dpool.tile([P, P], f32, name="dg")
                    nc.vector.tensor_scalar_mul(dg[:], ident[:], w[:, n : n + 1])
                    nc.tensor.matmul(
                        out=acc[:],
                        lhsT=dg[:],
                        rhs=F3[:, n, :],
                        start=(n == 0),
                        stop=(n == NC1 - 1),
                    )
                o = spool.tile([P, H], f32)
                nc.scalar.copy(o[:], acc[:])
                nc.sync.dma_start(out=out_d[rows, :], in_=o[:])

    nc.compile()
    return nc


def _get_nc():
    if "nc" not in _CACHE:
        _CACHE["nc"] = _build_nc()
    return _CACHE["nc"]


def _ensure_axon_hooks():
    """Provide antenv.axon_hooks if the image lacks it, so trace=True /
    BASS_TRACE=1 profiling requests don't crash run_bass_kernel_spmd."""
    import sys
    import types

    try:
        import antenv.axon_hooks  # noqa: F401

        return
    except ImportError:
        pass
    try:
        import antenv
    except ImportError:
        return
    mod = types.ModuleType("antenv.axon_hooks")
    state = {"hook": None}

    def set_axon_ntff_profile_hook(h):
        state["hook"] = h

    def get_axon_ntff_profile_hook():
        if state["hook"] is None:
            try:
                from trn_agent_boot.trn_boot import _ntff_profile_via_ctypes

                state["hook"] = _ntff_profile_via_ctypes("/opt/axon/libaxon_pjrt.so")
            except Exception:
                return None
        return state["hook"]

    mod.set_axon_ntff_profile_hook = set_axon_ntff_profile_hook
    mod.get_axon_ntff_profile_hook = get_axon_ntff_profile_hook
    sys.modules["antenv.axon_hooks"] = mod
    antenv.axon_hooks = mod


def kernel(**inputs) -> np.ndarray:
    _ensure_axon_hooks()
    from concourse.bass_utils import run_bass_kernel_spmd

    node_ids = np.ascontiguousarray(
        np.asarray(inputs["node_ids"]).astype(np.int32).reshape(B, S, 1)
    )
    neighs = np.ascontiguousarray(
        np.asarray(inputs["neighs"]).astype(np.int32).reshape(B, S, N)
    )
    mask = np.ascontiguousarray(
        np.asarray(inputs["mask"]).astype(np.int32).reshape(B, S, N)
    )
    emb = np.ascontiguousarray(np.asarray(inputs["emb_table"], dtype=np.float32))
    a_w = np.ascontiguousarray(
        np.asarray(inputs["a_w"], dtype=np.float32).reshape(2, H)
    )
    a_b = np.ascontiguousarray(
        np.asarray(inputs["a_b"], dtype=np.float32).reshape(1, 1)
    )

    nc = _get_nc()
    in_maps = [
        {
            "node_ids": node_ids[c],
            "neighs": neighs[c],
            "mask": mask[c],
            "emb_table": emb,
            "a_w": a_w,
            "a_b": a_b,
        }
        for c in range(N_CORES)
    ]
    res = run_bass_kernel_spmd(nc, in_maps, core_ids=list(range(N_CORES)))
    _CACHE["last_res"] = res
    out = np.stack([res.results[c]["out"] for c in range(N_CORES)], axis=0)
    return out.astype(np.float32)


# revision 10
# speedup vs baseline: 1.3837x; 1.3837x over previous
"""GAT message-passing kernel for Trainium2 (8 NeuronCores, SPMD).

Problem (per full input):
    B=8, S=512, N=32 neighbors, H=256, V=100001
    out[b,s,:] = sum_n softmax_n(leakyrelu(a_w . [src, cand_n]) + mask*NEG) * cand_n
    candidates = [self] + 32 neighbors (self never masked)

Sharding: data-parallel over B — core c handles batch row c with a
replicated embedding table.

Per-core algorithm (s-tiles of 128 nodes, 4 tiles):
    - build candidate id tile [128, 33] (col 0 = self, 1..32 = neighbors)
    - ONE indirect DMA gathers 33 rows of 256 f32 per partition:
      F[s][n,h], rows contiguous per partition (DMA-friendly 1KB rows)
    - logits: z[s,n] = a_src.src[s] + a_cand.cand[s,n] + a_b computed as
      fused tensor_tensor_reduce ops (mult + add-reduce with init scalar)
    - leakyrelu, neighbor masking, softmax over 33 slots
      (exp + denominator in ONE ScalarE op via accum_out)
    - weighted sum on the TensorEngine: out = sum_n diag(w[:,n]) @ F_n
      accumulated in PSUM across the 33 candidate slots
"""

import numpy as np

B, S, N, H, V = 8, 512, 32, 256, 100001
NC1 = N + 1  # 33 candidate slots (self + neighbors)
P = 128
S_TILES = S // P
NEG = -1.0e9
SLOPE = 0.2
N_CORES = 8

# Tuning knobs
F_BUFS = 3  # gather-tile buffering

_CACHE: dict = {}


def _build_nc():
    import concourse.bacc as bacc
    import concourse.mybir as mybir
    import concourse.tile as tile
    from concourse import bass
    from concourse.masks import make_identity

    f32 = mybir.dt.float32
    i32 = mybir.dt.int32
    Alu = mybir.AluOpType
    Act = mybir.ActivationFunctionType
    X = mybir.AxisListType.X

    nc = bacc.Bacc(
        "TRN2",
        target_bir_lowering=False,
        debug=False,
        enable_asserts=False,
        num_devices=N_CORES,
    )

    ids_d = nc.dram_tensor("node_ids", [S, 1], i32, kind="ExternalInput").ap()
    nbr_d = nc.dram_tensor("neighs", [S, N], i32, kind="ExternalInput").ap()
    msk_d = nc.dram_tensor("mask", [S, N], i32, kind="ExternalInput").ap()
    emb_d = nc.dram_tensor("emb_table", [V, H], f32, kind="ExternalInput").ap()
    aw_d = nc.dram_tensor("a_w", [2, H], f32, kind="ExternalInput").ap()
    ab_d = nc.dram_tensor("a_b", [1, 1], f32, kind="ExternalInput").ap()
    out_d = nc.dram_tensor("out", [S, H], f32, kind="ExternalOutput").ap()

    with tile.TileContext(nc) as tc:
        with (
            tc.tile_pool(name="cpool", bufs=1) as cpool,
            tc.tile_pool(name="fpool", bufs=F_BUFS) as fpool,
            tc.tile_pool(name="spool", bufs=2) as spool,
            tc.tile_pool(name="dpool", bufs=8) as dpool,
            tc.tile_pool(name="ppool", bufs=2, space="PSUM") as ppool,
        ):
            # ---- constants (once) ----
            ident = cpool.tile([P, P], f32)
            make_identity(nc, ident)

            # replicate a_w rows (a_src = row 0, a_cand = row 1) to all
            # 128 partitions with a tiny indirect gather
            # one offset (row 0) per partition; 512 contiguous floats cover
            # both a_w rows -> [aws | awc] per partition
            aw_idx = cpool.tile([P, 1], i32)
            nc.gpsimd.memset(aw_idx[:], 0)
            aw_rep = cpool.tile([P, 2 * H], f32)
            nc.gpsimd.indirect_dma_start(
                out=aw_rep[:],
                out_offset=None,
                in_=aw_d,
                in_offset=bass.IndirectOffsetOnAxis(ap=aw_idx[:], axis=0),
            )
            aws_rep = aw_rep[:, 0:H]
            awc_rep = aw_rep[:, H : 2 * H]

            ab_idx = cpool.tile([P, 1], i32)
            nc.gpsimd.memset(ab_idx[:], 0)
            ab_rep = cpool.tile([P, 1], f32)
            nc.gpsimd.indirect_dma_start(
                out=ab_rep[:],
                out_offset=None,
                in_=ab_d,
                in_offset=bass.IndirectOffsetOnAxis(ap=ab_idx[:], axis=0),
            )

            for t in range(S_TILES):
                rows = slice(t * P, (t + 1) * P)

                idx = spool.tile([P, NC1], i32)
                nc.sync.dma_start(out=idx[:, 0:1], in_=ids_d[rows, :])
                nc.sync.dma_start(out=idx[:, 1:NC1], in_=nbr_d[rows, :])
                mask_i = spool.tile([P, N], i32)
                nc.sync.dma_start(out=mask_i[:], in_=msk_d[rows, :])

                F = fpool.tile([P, NC1 * H], f32)
                F3 = F.rearrange("p (n h) -> p n h", n=NC1)
                # z[s,n] = sum_h F[s,n,h]*awc[h] + (sum_h F[s,0,h]*aws[h] + ab)
                # One gather per candidate slot (HW: one offset per partition
                # per indirect DMA); fused mul+accum per slot right behind it.
                trash = spool.tile([P, H], f32)
                zsrc = spool.tile([P, 1], f32)
                z = spool.tile([P, NC1], f32)
                for n in range(NC1):
                    nc.gpsimd.indirect_dma_start(
                        out=F3[:, n, :],
                        out_offset=None,
                        in_=emb_d,
                        in_offset=bass.IndirectOffsetOnAxis(
                            ap=idx[:, n : n + 1], axis=0
                        ),
                    )
                    if n == 0:
                        trash2 = spool.tile([P, H], f32)
                        nc.vector.scalar_tensor_tensor(
                            out=trash2[:],
                            in0=F3[:, 0, :],
                            scalar=1.0,
                            in1=aws_rep,
                            op0=Alu.mult,
                            op1=Alu.mult,
                            accum_out=zsrc[:],
                        )
                    nc.vector.scalar_tensor_tensor(
                        out=trash[:],
                        in0=F3[:, n, :],
                        scalar=1.0,
                        in1=awc_rep,
                        op0=Alu.mult,
                        op1=Alu.mult,
                        accum_out=z[:, n : n + 1],
                    )
                nc.vector.tensor_scalar_add(zsrc[:], zsrc[:], ab_rep[:])
                nc.vector.tensor_scalar_add(z[:], z[:], zsrc[:])

                # ---- leakyrelu ----
                z2 = spool.tile([P, NC1], f32)
                nc.vector.tensor_scalar_mul(z2[:], z[:], SLOPE)
                nc.vector.tensor_max(z[:], z[:], z2[:])

                # ---- mask neighbors: z[:,1:] += mask * NEG ----
                mask_f = spool.tile([P, N], f32)
                nc.vector.tensor_copy(mask_f[:], mask_i[:])
                nc.vector.scalar_tensor_tensor(
                    out=z[:, 1:NC1],
                    in0=mask_f[:],
                    scalar=NEG,
                    in1=z[:, 1:NC1],
                    op0=Alu.mult,
                    op1=Alu.add,
                )

                # ---- softmax over 33 slots ----
                negm = spool.tile([P, 1], f32)
                nc.vector.tensor_reduce(
                    negm[:], z[:], axis=X, op=Alu.max, negate=True
                )
                e = spool.tile([P, NC1], f32)
                den = spool.tile([P, 1], f32)
                nc.scalar.activation(
                    e[:], z[:], Act.Exp, bias=negm[:], scale=1.0, accum_out=den[:]
                )
                rden = spool.tile([P, 1], f32)
                nc.vector.reciprocal(rden[:], den[:])
                w = spool.tile([P, NC1], f32)
                nc.vector.tensor_scalar_mul(w[:], e[:], rden[:])

                # ---- weighted aggregation on TensorE ----
                acc = ppool.tile([P, H], f32)
                for n in range(NC1):
                    dg = dpool.tile([P, P], f32, name="dg")
                    nc.vector.tensor_scalar_mul(dg[:], ident[:], w[:, n : n + 1])
                    nc.tensor.matmul(
                        out=acc[:],
                        lhsT=dg[:],
                        rhs=F3[:, n, :],
                        start=(n == 0),
                        stop=(n == NC1 - 1),
                    )
                o = spool.tile([P, H], f32)
                nc.scalar.copy(o[:], acc[:])
                nc.sync.dma_start(out=out_d[rows, :], in_=o[:])

    nc.compile()
    return nc


def _get_nc():
    if "nc" not in _CACHE:
        _CACHE["nc"] = _build_nc()
    return _CACHE["nc"]


def _ensure_axon_hooks():
    """Provide antenv.axon_hooks if the image lacks it, so trace=True /
    BASS_TRACE=1 profiling requests don't crash run_bass_kernel_spmd."""
    import sys
    import types

    try:
        import antenv.axon_hooks  # noqa: F401

        return
    except ImportError:
        pass
    try:
        import antenv
    except ImportError:
        return
    mod = types.ModuleType("antenv.axon_hooks")
    state = {"hook": None}

    def set_axon_ntff_profile_hook(h):
        state["hook"] = h

    def get_axon_ntff_profile_hook():
        if state["hook"] is None:
            try:
                from trn_agent_boot.trn_boot import _ntff_profile_via_ctypes

                state["hook"] = _ntff_profile_via_ctypes("/opt/axon/libaxon_pjrt.so")
            except Exception:
                return None
        return state["hook"]

    mod.set_axon_ntff_profile_hook = set_axon_ntff_profile_hook
    mod.get_axon_ntff_profile_hook = get_axon_ntff_profile_hook
    sys.modules["antenv.axon_hooks"] = mod
    antenv.axon_hooks = mod


def kernel(**inputs) -> np.ndarray:
    _ensure_axon_hooks()
    from concourse.bass_utils import run_bass_kernel_spmd

    node_ids = np.ascontiguousarray(
        np.asarray(inputs["node_ids"]).astype(np.int32).reshape(B, S, 1)
    )
    neighs = np.ascontiguousarray(
        np.asarray(inputs["neighs"]).astype(np.int32).reshape(B, S, N)
    )
    mask = np.ascontiguousarray(
        np.asarray(inputs["mask"]).astype(np.int32).reshape(B, S, N)
    )
    emb = np.ascontiguousarray(np.asarray(inputs["emb_table"], dtype=np.float32))
    a_w = np.ascontiguousarray(
        np.asarray(inputs["a_w"], dtype=np.float32).reshape(2, H)
    )
    a_b = np.ascontiguousarray(
        np.asarray(inputs["a_b"], dtype=np.float32).reshape(1, 1)
    )

    nc = _get_nc()
    in_maps = [
        {
            "node_ids": node_ids[c],
            "neighs": neighs[c],
            "mask": mask[c],
            "emb_table": emb,
            "a_w": a_w,
            "a_b": a_b,
        }
        for c in range(N_CORES)
    ]
    res = run_bass_kernel_spmd(nc, in_maps, core_ids=list(range(N_CORES)))
    _CACHE["last_res"] = res
    out = np.stack([res.results[c]["out"] for c in range(N_CORES)], axis=0)
    return out.astype(np.float32)


# revision 15
# speedup vs baseline: 1.4324x; 1.0352x over previous
"""GAT message-passing kernel for Trainium2 (8 NeuronCores, SPMD).

Problem (per full input):
    B=8, S=512, N=32 neighbors, H=256, V=100001
    out[b,s,:] = sum_n softmax_n(leakyrelu(a_w . [src, cand_n]) + mask*NEG) * cand_n
    candidates = [self] + 32 neighbors (self never masked)

Sharding: data-parallel over B — core c handles batch row c with a
replicated embedding table.

Per-core algorithm (s-tiles of 128 nodes, 4 tiles):
    - build candidate id tile [128, 33] (col 0 = self, 1..32 = neighbors)
    - ONE indirect DMA gathers 33 rows of 256 f32 per partition:
      F[s][n,h], rows contiguous per partition (DMA-friendly 1KB rows)
    - logits: z[s,n] = a_src.src[s] + a_cand.cand[s,n] + a_b computed as
      fused tensor_tensor_reduce ops (mult + add-reduce with init scalar)
    - leakyrelu, neighbor masking, softmax over 33 slots
      (exp + denominator in ONE ScalarE op via accum_out)
    - weighted sum on the TensorEngine: out = sum_n diag(w[:,n]) @ F_n
      accumulated in PSUM across the 33 candidate slots
"""

import numpy as np

B, S, N, H, V = 8, 512, 32, 256, 100001
NC1 = N + 1  # 33 candidate slots (self + neighbors)
P = 128
S_TILES = S // P
NEG = -1.0e9
SLOPE = 0.2
N_CORES = 8

# Tuning knobs
F_BUFS = 3  # gather-tile buffering

_CACHE: dict = {}


def _build_nc():
    import concourse.bacc as bacc
    import concourse.mybir as mybir
    import concourse.tile as tile
    from concourse import bass
    from concourse.masks import make_identity

    f32 = mybir.dt.float32
    bf16 = mybir.dt.bfloat16
    i32 = mybir.dt.int32
    Alu = mybir.AluOpType
    Act = mybir.ActivationFunctionType
    X = mybir.AxisListType.X

    nc = bacc.Bacc(
        "TRN2",
        target_bir_lowering=False,
        debug=False,
        enable_asserts=False,
        num_devices=N_CORES,
    )

    ids_d = nc.dram_tensor("node_ids", [S, 1], i32, kind="ExternalInput").ap()
    nbr_d = nc.dram_tensor("neighs", [S, N], i32, kind="ExternalInput").ap()
    msk_d = nc.dram_tensor("mask", [S, N], i32, kind="ExternalInput").ap()
    emb_d = nc.dram_tensor("emb_table", [V, H], f32, kind="ExternalInput").ap()
    aw_d = nc.dram_tensor("a_w", [2, H], f32, kind="ExternalInput").ap()
    ab_d = nc.dram_tensor("a_b", [1, 1], f32, kind="ExternalInput").ap()
    out_d = nc.dram_tensor("out", [S, H], f32, kind="ExternalOutput").ap()

    with tile.TileContext(nc) as tc:
        with (
            tc.tile_pool(name="cpool", bufs=1) as cpool,
            tc.tile_pool(name="fpool", bufs=F_BUFS) as fpool,
            tc.tile_pool(name="spool", bufs=2) as spool,
            tc.tile_pool(name="dpool", bufs=8) as dpool,
            tc.tile_pool(name="ppool", bufs=2, space="PSUM") as ppool,
        ):
            # ---- constants (once) ----
            ident = cpool.tile([P, P], f32)
            make_identity(nc, ident)

            # replicate a_w rows (a_src = row 0, a_cand = row 1) to all
            # 128 partitions with a tiny indirect gather
            # one offset (row 0) per partition; 512 contiguous floats cover
            # both a_w rows -> [aws | awc] per partition
            aw_idx = cpool.tile([P, 1], i32)
            nc.gpsimd.memset(aw_idx[:], 0)
            aw_rep = cpool.tile([P, 2 * H], f32)
            nc.gpsimd.indirect_dma_start(
                out=aw_rep[:],
                out_offset=None,
                in_=aw_d,
                in_offset=bass.IndirectOffsetOnAxis(ap=aw_idx[:], axis=0),
            )
            aws_rep = aw_rep[:, 0:H]
            awc_rep = aw_rep[:, H : 2 * H]

            ab_idx = cpool.tile([P, 1], i32)
            nc.gpsimd.memset(ab_idx[:], 0)
            ab_rep = cpool.tile([P, 1], f32)
            nc.gpsimd.indirect_dma_start(
                out=ab_rep[:],
                out_offset=None,
                in_=ab_d,
                in_offset=bass.IndirectOffsetOnAxis(ap=ab_idx[:], axis=0),
            )

            for t in range(S_TILES):
                rows = slice(t * P, (t + 1) * P)

                idx = spool.tile([P, NC1], i32)
                nc.sync.dma_start(out=idx[:, 0:1], in_=ids_d[rows, :])
                nc.sync.dma_start(out=idx[:, 1:NC1], in_=nbr_d[rows, :])
                mask_i = spool.tile([P, N], i32)
                nc.sync.dma_start(out=mask_i[:], in_=msk_d[rows, :])

                F = fpool.tile([P, NC1 * H], f32)
                F3 = F.rearrange("p (n h) -> p n h", n=NC1)
                Fb = fpool.tile([P, NC1 * H], bf16, name="Fb")
                Fb3 = Fb.rearrange("p (n h) -> p n h", n=NC1)
                # z[s,n] = sum_h F[s,n,h]*awc[h] + (sum_h F[s,0,h]*aws[h] + ab)
                # One gather per candidate slot (HW: one offset per partition
                # per indirect DMA); fused mul+accum per slot right behind it.
                trash = spool.tile([P, H], f32)
                zsrc = spool.tile([P, 1], f32)
                z = spool.tile([P, NC1], f32)
                for n in range(NC1):
                    nc.gpsimd.indirect_dma_start(
                        out=F3[:, n, :],
                        out_offset=None,
                        in_=emb_d,
                        in_offset=bass.IndirectOffsetOnAxis(
                            ap=idx[:, n : n + 1], axis=0
                        ),
                    )
                    if n == 0:
                        trash2 = spool.tile([P, H], f32)
                        nc.vector.scalar_tensor_tensor(
                            out=trash2[:],
                            in0=F3[:, 0, :],
                            scalar=1.0,
                            in1=aws_rep,
                            op0=Alu.mult,
                            op1=Alu.mult,
                            accum_out=zsrc[:],
                        )
                    nc.vector.scalar_tensor_tensor(
                        out=trash[:],
                        in0=F3[:, n, :],
                        scalar=1.0,
                        in1=awc_rep,
                        op0=Alu.mult,
                        op1=Alu.mult,
                        accum_out=z[:, n : n + 1],
                    )
                    # bf16 copy feeds the TensorE aggregation (1-pass matmul)
                    nc.scalar.copy(Fb3[:, n, :], F3[:, n, :])
                nc.vector.tensor_scalar_add(zsrc[:], zsrc[:], ab_rep[:])
                nc.vector.tensor_scalar_add(z[:], z[:], zsrc[:])

                # ---- leakyrelu ----
                z2 = spool.tile([P, NC1], f32)
                nc.vector.tensor_scalar_mul(z2[:], z[:], SLOPE)
                nc.vector.tensor_max(z[:], z[:], z2[:])

                # ---- mask neighbors: z[:,1:] += mask * NEG ----
                mask_f = spool.tile([P, N], f32)
                nc.vector.tensor_copy(mask_f[:], mask_i[:])
                nc.vector.scalar_tensor_tensor(
                    out=z[:, 1:NC1],
                    in0=mask_f[:],
                    scalar=NEG,
                    in1=z[:, 1:NC1],
                    op0=Alu.mult,
                    op1=Alu.add,
                )

                # ---- softmax over 33 slots ----
                negm = spool.tile([P, 1], f32)
                nc.vector.tensor_reduce(
                    negm[:], z[:], axis=X, op=Alu.max, negate=True
                )
                e = spool.tile([P, NC1], f32)
                den = spool.tile([P, 1], f32)
                nc.scalar.activation(
                    e[:], z[:], Act.Exp, bias=negm[:], scale=1.0, accum_out=den[:]
                )
                rden = spool.tile([P, 1], f32)
                nc.vector.reciprocal(rden[:], den[:])
                w = spool.tile([P, NC1], f32)
                nc.vector.tensor_scalar_mul(w[:], e[:], rden[:])

                # ---- weighted aggregation on TensorE ----
                acc = ppool.tile([P, H], f32)
                for n in range(NC1):
                    dg = dpool.tile([P, P], bf16, name="dg")
                    nc.vector.tensor_scalar_mul(dg[:], ident[:], w[:, n : n + 1])
                    nc.tensor.matmul(
                        out=acc[:],
                        lhsT=dg[:],
                        rhs=Fb3[:, n, :],
                        start=(n == 0),
                        stop=(n == NC1 - 1),
                    )
                o = spool.tile([P, H], f32)
                nc.scalar.copy(o[:], acc[:])
                nc.sync.dma_start(out=out_d[rows, :], in_=o[:])

    nc.compile()
    return nc


def _get_nc():
    if "nc" not in _CACHE:
        _CACHE["nc"] = _build_nc()
    return _CACHE["nc"]


def _ensure_axon_hooks():
    """Provide antenv.axon_hooks if the image lacks it, so trace=True /
    BASS_TRACE=1 profiling requests don't crash run_bass_kernel_spmd."""
    import sys
    import types

    try:
        import antenv.axon_hooks  # noqa: F401

        return
    except ImportError:
        pass
    try:
        import antenv
    except ImportError:
        return
    mod = types.ModuleType("antenv.axon_hooks")
    state = {"hook": None}

    def set_axon_ntff_profile_hook(h):
        state["hook"] = h

    def get_axon_ntff_profile_hook():
        if state["hook"] is None:
            try:
                from trn_agent_boot.trn_boot import _ntff_profile_via_ctypes

                state["hook"] = _ntff_profile_via_ctypes("/opt/axon/libaxon_pjrt.so")
            except Exception:
                return None
        return state["hook"]

    mod.set_axon_ntff_profile_hook = set_axon_ntff_profile_hook
    mod.get_axon_ntff_profile_hook = get_axon_ntff_profile_hook
    sys.modules["antenv.axon_hooks"] = mod
    antenv.axon_hooks = mod


def kernel(**inputs) -> np.ndarray:
    _ensure_axon_hooks()
    from concourse.bass_utils import run_bass_kernel_spmd

    node_ids = np.ascontiguousarray(
        np.asarray(inputs["node_ids"]).astype(np.int32).reshape(B, S, 1)
    )
    neighs = np.ascontiguousarray(
        np.asarray(inputs["neighs"]).astype(np.int32).reshape(B, S, N)
    )
    mask = np.ascontiguousarray(
        np.asarray(inputs["mask"]).astype(np.int32).reshape(B, S, N)
    )
    emb = np.ascontiguousarray(np.asarray(inputs["emb_table"], dtype=np.float32))
    a_w = np.ascontiguousarray(
        np.asarray(inputs["a_w"], dtype=np.float32).reshape(2, H)
    )
    a_b = np.ascontiguousarray(
        np.asarray(inputs["a_b"], dtype=np.float32).reshape(1, 1)
    )

    nc = _get_nc()
    in_maps = [
        {
            "node_ids": node_ids[c],
            "neighs": neighs[c],
            "mask": mask[c],
            "emb_table": emb,
            "a_w": a_w,
            "a_b": a_b,
        }
        for c in range(N_CORES)
    ]
    res = run_bass_kernel_spmd(nc, in_maps, core_ids=list(range(N_CORES)))
    _CACHE["last_res"] = res
    out = np.stack([res.results[c]["out"] for c in range(N_CORES)], axis=0)
    return out.astype(np.float32)
